# revision 16
# baseline (speedup 1.0000x reference)
"""Trainium2 Bass kernel for nn_MinkHead (3-level FPN with gather-based
transposed convs, gather-GEMM form).

Strategy (8 NeuronCores, SPMD, no collectives):
  - Level-3 (coarsest) rows are sharded contiguously: core c owns rows
    [c*S3, (c+1)*S3).
  - Each level-2 point j is assigned to the core owning parent2[j]; each
    level-1 point i is assigned to the core owning its parent's level-2
    point.  All parent gathers are then core-local.
  - Within a core, points are grouped by tconv offset k (8 groups, padded
    to a common capacity G) so each group's tconv is a clean GEMM with
    T[k].
  - On-chip layout is feature-major ([C, points]) for matmul lhsT;
    intermediate features y3/y2 are kept in SBUF as bf16 row-major
    "tokens" (token i at partition i%128, 256B stripe i//128) which is
    both what the PSUM->SBUF copy naturally produces and what the
    SBUF-source dma_gather consumes.  Gather output is [C, points] bf16,
    directly usable as the next matmul's lhsT.  No transposes anywhere.
  - The 1x1 conv path stays fp32; only the tconv gather path is bf16.

Host side: shard/permute/pad inputs per core (pure numpy), run the SPMD
program via run_bass_kernel_spmd, scatter the per-core outputs back into
the full [N1, 128] output.
"""

import numpy as np
import ml_dtypes

import concourse.bass as bass
import concourse.mybir as mybir
import concourse.tile as tile
from concourse import bacc
from concourse.bass import ts
from concourse.bass_utils import run_bass_kernel_spmd

F32 = mybir.dt.float32
BF16 = mybir.dt.bfloat16
I16 = mybir.dt.int16
BF16_NP = ml_dtypes.bfloat16

NCORES = 8
KVOL = 8
C1, C2, C3, D = 64, 128, 256, 128
LEVEL1_CH = 8       # level-1 tiles (128 pts) per gather/DMA chunk
# single_packet=True crashes the device (NRT unrecoverable) for gathers
# with num_idxs > ~128; packetized mode works at any size.
SINGLE_PACKET = False


def _ru(x, m):
    return (x + m - 1) // m * m


# ---------------------------------------------------------------------------
# Host-side sharding / permutation plan
# ---------------------------------------------------------------------------

def _plan(feats1, feats2, feats3, parent1, offset1, parent2, offset2,
          W1, W2, W3, T2, T3):
    N1 = feats1.shape[0]
    N2 = feats2.shape[0]
    N3 = feats3.shape[0]
    S3 = _ru(N3, NCORES) // NCORES
    M3pad = _ru(S3, 128)

    own2 = (parent2 // S3).astype(np.int32)
    own1 = own2[parent1]

    def group_layout(own, off):
        # rank of each point within its (core, offset) group, group sizes
        key = own * KVOL + off
        counts = np.bincount(key, minlength=NCORES * KVOL)
        order = np.argsort(key, kind='stable')
        ranks = np.empty(len(key), np.int64)
        starts = np.zeros(NCORES * KVOL, np.int64)
        starts[1:] = np.cumsum(counts)[:-1]
        ranks[order] = np.arange(len(key)) - starts[key[order]]
        return ranks, counts

    ranks2, counts2 = group_layout(own2, offset2)
    G2 = _ru(int(counts2.max()), 128)
    P2 = KVOL * G2
    assert P2 < 32768, f"P2={P2} exceeds int16 gather index range"
    pos2 = (offset2.astype(np.int64) * G2 + ranks2)  # position in owner's list

    ranks1, counts1 = group_layout(own1, offset1)
    G1 = _ru(int(counts1.max()), 128)
    P1 = KVOL * G1
    pos1 = (offset1.astype(np.int64) * G1 + ranks1)

    lp2_all = (parent2 - own2 * S3).astype(np.int16)      # local y3 token
    lp1_all = pos2[parent1].astype(np.int16)              # local y2 token

    def wrap_idx(a):
        # int16 [P] -> [128, P//16]: index i at partition i%16, col i//16,
        # replicated across the 8 gpsimd core groups.
        w = a.reshape(-1, 16).T
        return np.ascontiguousarray(np.tile(w, (8, 1)))

    f32 = np.float32
    in_maps = []
    idx1_cores = []
    nreal_cores = []
    for c in range(NCORES):
        sel2 = own2 == c
        p2l = pos2[sel2]
        lp2 = np.zeros(P2, np.int16)
        lp2[p2l] = lp2_all[sel2]
        f2T = np.zeros((C2, P2), f32)
        f2T[:, p2l] = feats2[sel2].T

        sel1 = own1 == c
        p1l = pos1[sel1]
        lp1 = np.zeros(P1, np.int16)
        lp1[p1l] = lp1_all[sel1]
        idx1 = np.full(P1, -1, np.int64)
        idx1[p1l] = np.nonzero(sel1)[0]
        f1full = np.zeros((P1, C1), f32)
        f1full[p1l] = feats1[sel1]
        # pack two point-halves vertically: [128, P1//2]
        f1P = np.ascontiguousarray(
            np.concatenate([f1full[:P1 // 2].T, f1full[P1 // 2:].T], axis=0))

        r3 = min(S3, N3 - c * S3)
        f3T = np.zeros((128, 2, M3pad), f32)
        blk = feats3[c * S3:c * S3 + r3]  # [r3, 256]
        f3T[:, 0, :r3] = blk[:, :128].T
        f3T[:, 1, :r3] = blk[:, 128:].T

        in_maps.append({
            "f3T": f3T,
            "f2T": f2T,
            "f1P": f1P,
            "i2": wrap_idx(lp2),
            "i1": wrap_idx(lp1),
        })
        idx1_cores.append(idx1)
        nreal_cores.append(int(sel1.sum()))

    # replicated weights
    w3 = np.ascontiguousarray(
        W3.reshape(2, 128, D).transpose(1, 0, 2)).astype(f32)  # [128,2,128]
    w1s = np.concatenate([W1, W1], axis=0).astype(f32)          # [128,128]
    t3b = np.ascontiguousarray(T3.transpose(1, 0, 2)).astype(BF16_NP)  # [128,8,128]
    t2b = np.ascontiguousarray(T2.transpose(1, 0, 2)).astype(BF16_NP)
    w2 = np.ascontiguousarray(W2.astype(f32))
    for m in in_maps:
        m.update({"w3": w3, "w2": w2, "w1s": w1s, "t3": t3b, "t2": t2b})

    return dict(M3pad=M3pad, G2=G2, G1=G1, P1=P1, N1=N1,
                in_maps=in_maps, idx1_cores=idx1_cores,
                nreal_cores=nreal_cores)


# ---------------------------------------------------------------------------
# Device program
# ---------------------------------------------------------------------------

def _build_nc(M3pad, G2, G1):
    R3 = M3pad // 128
    P2 = KVOL * G2
    R2 = P2 // 128
    P1 = KVOL * G1
    nt1 = G1 // 128  # level-1 tiles per offset group

    nc = bacc.Bacc("TRN2", target_bir_lowering=False, debug=False,
                   num_devices=NCORES)
    f3T = nc.declare_dram_parameter("f3T", [128, 2, M3pad], F32, isOutput=False)
    w3 = nc.declare_dram_parameter("w3", [128, 2, 128], F32, isOutput=False)
    f2T = nc.declare_dram_parameter("f2T", [128, P2], F32, isOutput=False)
    w2 = nc.declare_dram_parameter("w2", [128, 128], F32, isOutput=False)
    f1P = nc.declare_dram_parameter("f1P", [128, P1 // 2], F32, isOutput=False)
    w1s = nc.declare_dram_parameter("w1s", [128, 128], F32, isOutput=False)
    t3 = nc.declare_dram_parameter("t3", [128, KVOL, 128], BF16, isOutput=False)
    t2 = nc.declare_dram_parameter("t2", [128, KVOL, 128], BF16, isOutput=False)
    i2 = nc.declare_dram_parameter("i2", [128, P2 // 16], I16, isOutput=False)
    i1 = nc.declare_dram_parameter("i1", [128, P1 // 16], I16, isOutput=False)
    out = nc.declare_dram_parameter("out", [P1, 128], F32, isOutput=True)

    with tile.TileContext(nc) as tc:
        with (
            tc.tile_pool(name="const", bufs=1) as const,
            tc.tile_pool(name="ytok", bufs=1) as ytok,
            tc.tile_pool(name="f3pool", bufs=2) as f3pool,
            tc.tile_pool(name="f2pool", bufs=3) as f2pool,
            tc.tile_pool(name="f1pool", bufs=3) as f1pool,
            tc.tile_pool(name="g2pool", bufs=3) as g2pool,
            tc.tile_pool(name="g1pool", bufs=3) as g1pool,
            tc.tile_pool(name="opool", bufs=3) as opool,
            tc.tile_pool(name="psum", bufs=6, space="PSUM") as psum,
        ):
            w3t = const.tile([128, 2, 128], F32)
            nc.sync.dma_start(out=w3t[:], in_=w3[:])
            w2t = const.tile([128, 128], F32)
            nc.sync.dma_start(out=w2t[:], in_=w2[:])
            w1t = const.tile([128, 128], F32)
            nc.sync.dma_start(out=w1t[:], in_=w1s[:])
            t3t = const.tile([128, KVOL, 128], BF16)
            nc.sync.dma_start(out=t3t[:], in_=t3[:])
            t2t = const.tile([128, KVOL, 128], BF16)
            nc.sync.dma_start(out=t2t[:], in_=t2[:])
            i2t = const.tile([128, P2 // 16], I16)
            nc.sync.dma_start(out=i2t[:], in_=i2[:])
            i1t = const.tile([128, P1 // 16], I16)
            nc.sync.dma_start(out=i1t[:], in_=i1[:])

            y3tok = ytok.tile([128, R3, 128], BF16)
            y2tok = ytok.tile([128, R2, 128], BF16)

            # shared num_idxs registers (one per distinct gather size --
            # a fresh to_reg per gather exhausts the Pool register file)
            CH = LEVEL1_CH  # level-1: 128-col tiles per chunk
            rem = nt1 % CH
            nreg = {G2: nc.gpsimd.to_reg(G2)}
            if CH * 128 not in nreg:
                nreg[CH * 128] = nc.gpsimd.to_reg(CH * 128)
            if rem and rem * 128 not in nreg:
                nreg[rem * 128] = nc.gpsimd.to_reg(rem * 128)

            # ------------------ level 3: y3 = feats3 @ W3 ------------------
            CH3 = 4
            for c0 in range(0, R3, CH3):
                n = min(CH3, R3 - c0)
                f3c = f3pool.tile([128, 2, CH3 * 128], F32, tag="f3c")
                nc.sync.dma_start(
                    out=f3c[:, :, :n * 128],
                    in_=f3T[:, :, c0 * 128:(c0 + n) * 128])
                for j in range(n):
                    t = c0 + j
                    ps = psum.tile([128, 128], F32, tag="ps")
                    nc.tensor.matmul(ps[:], f3c[:, 0, ts(j, 128)],
                                     w3t[:, 0, :], start=True, stop=False)
                    nc.tensor.matmul(ps[:], f3c[:, 1, ts(j, 128)],
                                     w3t[:, 1, :], start=False, stop=True)
                    nc.vector.tensor_copy(y3tok[:, t, :], ps[:])

            # --------- level 2: y2 = gather(y3) @ T3[k] + feats2 @ W2 -------
            nt2 = G2 // 128
            for k in range(KVOL):
                g2c = g2pool.tile([128, 1, G2], BF16, tag="g2c")
                nc.gpsimd.dma_gather(
                    out_ap=g2c[:],
                    in_ap=y3tok[:],
                    idxs_ap=i2t[:, ts(k, G2 // 16)],
                    num_idxs=G2,
                    num_idxs_reg=nreg[G2],
                    elem_size=128,
                    transpose=True,
                    sbuf_tokens_per_rank=128,
                    sbuf_free_dim_per_rank=256,
                    single_packet=SINGLE_PACKET,
                )
                f2c = f2pool.tile([128, G2], F32, tag="f2c")
                nc.sync.dma_start(out=f2c[:], in_=f2T[:, ts(k, G2)])
                for t in range(nt2):
                    ps2 = psum.tile([128, 128], F32, tag="ps")
                    nc.tensor.matmul(ps2[:], f2c[:, ts(t, 128)], w2t[:],
                                     start=True, stop=False)
                    nc.tensor.matmul(ps2[:], g2c[:, 0, ts(t, 128)],
                                     t3t[:, k, :], start=False, stop=True)
                    nc.vector.tensor_copy(y2tok[:, k * nt2 + t, :], ps2[:])

            # --- level 1: out = gather(y2) @ T2[k] + feats1 @ W1 (2 halves) -
            for kt in range(4):
                for c0 in range(0, nt1, CH):
                    n = min(CH, nt1 - c0)
                    gA = g1pool.tile([128, 1, CH * 128], BF16, tag="gA")
                    gB = g1pool.tile([128, 1, CH * 128], BF16, tag="gB")
                    nc.gpsimd.dma_gather(
                        out_ap=gA[:, :, :n * 128],
                        in_ap=y2tok[:],
                        idxs_ap=i1t[:, (kt * G1 + c0 * 128) // 16:
                                    (kt * G1 + (c0 + n) * 128) // 16],
                        num_idxs=n * 128,
                        num_idxs_reg=nreg[n * 128],
                        elem_size=128,
                        transpose=True,
                        sbuf_tokens_per_rank=128,
                        sbuf_free_dim_per_rank=256,
                        single_packet=SINGLE_PACKET,
                    )
                    nc.gpsimd.dma_gather(
                        out_ap=gB[:, :, :n * 128],
                        in_ap=y2tok[:],
                        idxs_ap=i1t[:, ((kt + 4) * G1 + c0 * 128) // 16:
                                    ((kt + 4) * G1 + (c0 + n) * 128) // 16],
                        num_idxs=n * 128,
                        num_idxs_reg=nreg[n * 128],
                        elem_size=128,
                        transpose=True,
                        sbuf_tokens_per_rank=128,
                        sbuf_free_dim_per_rank=256,
                        single_packet=SINGLE_PACKET,
                    )
                    f1c = f1pool.tile([128, CH * 128], F32, tag="f1c")
                    nc.sync.dma_start(
                        out=f1c[:, :n * 128],
                        in_=f1P[:, kt * G1 + c0 * 128:kt * G1 + (c0 + n) * 128])
                    oA = opool.tile([128, CH, 128], F32, tag="oA")
                    oB = opool.tile([128, CH, 128], F32, tag="oB")
                    for j in range(n):
                        psA = psum.tile([128, 128], F32, tag="ps")
                        psB = psum.tile([128, 128], F32, tag="ps")
                        nc.tensor.matmul(psA[:], f1c[0:64, ts(j, 128)],
                                         w1t[0:64, :], start=True, stop=False)
                        nc.tensor.matmul(psA[:], gA[:, 0, ts(j, 128)],
                                         t2t[:, kt, :], start=False, stop=True)
                        nc.tensor.matmul(psB[:], f1c[64:128, ts(j, 128)],
                                         w1t[64:128, :], start=True, stop=False)
                        nc.tensor.matmul(psB[:], gB[:, 0, ts(j, 128)],
                                         t2t[:, kt + 4, :], start=False, stop=True)
                        nc.vector.tensor_copy(oA[:, j, :], psA[:])
                        nc.vector.tensor_copy(oB[:, j, :], psB[:])
                    outA = out[kt * G1 + c0 * 128:kt * G1 + (c0 + n) * 128, :]
                    nc.sync.dma_start(
                        out=outA.rearrange("(t p) d -> p t d", p=128),
                        in_=oA[:, :n, :])
                    outB = out[(kt + 4) * G1 + c0 * 128:
                               (kt + 4) * G1 + (c0 + n) * 128, :]
                    nc.sync.dma_start(
                        out=outB.rearrange("(t p) d -> p t d", p=128),
                        in_=oB[:, :n, :])
    nc.compile()
    return nc


# ---------------------------------------------------------------------------
# Entry point
# ---------------------------------------------------------------------------

def prepare(inputs):
    plan = _plan(**inputs)
    nc = _build_nc(plan["M3pad"], plan["G2"], plan["G1"])
    return nc, plan


def assemble(plan, results):
    out = np.zeros((plan["N1"], D), np.float32)
    for c in range(NCORES):
        idx1 = plan["idx1_cores"][c]
        real = idx1 >= 0
        out[idx1[real]] = results[c]["out"][real]
    return out


def kernel(**inputs):
    inputs = {k: np.asarray(v) for k, v in inputs.items()}
    nc, plan = prepare(inputs)
    res = run_bass_kernel_spmd(nc, plan["in_maps"],
                               core_ids=list(range(NCORES)))
    return assemble(plan, res.results)
